# revision 53
# baseline (speedup 1.0000x reference)
"""Trainium2 Bass kernel for nn_Attention_36404142801494.

Fused causal self-attention (q=k=v=Wq(x)) + output projection, sharded over
8 NeuronCores: data-parallel on batch (B=2 -> 2 groups of 4 cores), tensor-
parallel on heads (8 heads -> 2 heads/core, dq = 128 hidden dims/core) with a
column-split Wq and a row-split Wo. Each core returns a partial [S, HID]
output (bf16); the host sums the 4 partials per batch and adds the Wo bias.

Per-core structure (keyed to the TimelineSim cost model, where a matmul costs
out_free_size x cycles_per_row and LDWEIGHTS is free):
  - qT [d=128, s] f32r via Wq matmuls (moving = x blocks, N=512/256).
  - QK emits scoresT tiles [k=128, q<=512] per key-chunk, causally trimmed;
    f32r keeps 1 cyc/row at N>=256 (diagonal chunks padded to N=256 min).
  - exp on ACT into bf16 et tiles; diagonal 128x128 blocks masked by a
    lower-triangular bf16 mask multiply on DVE (mask generated on device).
  - AV runs TRANSPOSED: stationary = et 128x128 block, moving = V chunk
    [k=128, 65] bf16 (col 64 = ones) -> av psum [q=128, 65], N=65/matmul.
    Col 64 accumulates the softmax denominator for free; normalization is a
    per-partition reciprocal + tensor_scalar multiply (no DRAM bounce).
  - ao [q, d] bf16 is PE-transposed (bf16 identity) to aoT [d, q]; both
    heads stack to [128, q] so Wo is one K=128 matmul per 128-query chunk
    (moving = WoT [128 d, 512 c] bf16, N=512).
  - Output partials DMA out as bf16 [2048, 512], split across HWDGE and
    SWDGE queues.

Schedule: unit (h, qb) = the QK/exp stream for one head x query block.
Units run (0,0),(1,0),(0,1),(1,1),(0,2),(0,3),(1,3),(1,2); qproj of the
next block is emitted as housekeeping inside an earlier unit so the ACT
engine keeps exp backlog across block boundaries. Heads 1 of qb3 and qb2
drain through software-pipelined per-query-chunk wavefronts (AV -> recip/
mul -> transpose -> copy -> Wo -> DMA); the qb3 drain overlaps unit (1,2)'s
exp stream and the final qb2 drain reuses the freed QK psum banks. A block
of dependency-free PE warmup transposes before qproj(0) beats the p-state
ramp while the first DMAs land. Inputs x/Wq/Wo stream in as bf16.

Everything is hardcoded for B=2, S=2048, HID=512, NH=8, HD=64.
"""

import sys

sys.path.insert(0, "/opt/trn_rl_repo")

import numpy as np
import ml_dtypes

import concourse.bass as bass
import concourse.bacc as bacc
import concourse.tile as tile
import concourse.mybir as mybir
from concourse.bass_utils import run_bass_kernel_spmd

f32 = mybir.dt.float32
f32r = mybir.dt.float32r
bf16 = mybir.dt.bfloat16

B, S, HID = 2, 2048, 512
NH, HD = 8, 64
N_CORES = 8
SB = 512
SCALE = 1.0 / np.sqrt(HD)
N_WARMUP = 22

Exp = mybir.ActivationFunctionType.Exp
ALU = mybir.AluOpType


def build_nc():
    nc = bacc.Bacc(None, target_bir_lowering=False)

    # host pre-arranged layouts (see make_in_maps):
    #   xB[p, i, s]  = x[b].T[128*i + p, s]
    #   WqB[p, i, c] = Wq_w[dq, :].T[128*i + p, c]
    #   WoT[d, c]    = Wo_w[:, dq].T[d, c]
    xB = nc.dram_tensor("xB", [128, 4, S], bf16, kind="ExternalInput")
    WqB = nc.dram_tensor("WqB", [128, 513], bf16, kind="ExternalInput")
    WoT = nc.dram_tensor("WoT", [128, HID], bf16, kind="ExternalInput")
    out_part = nc.dram_tensor("out_part", [S, HID], bf16, kind="ExternalOutput")

    with tile.TileContext(nc) as tc:
        with (
            tc.tile_pool(name="singles", bufs=1) as singles,
            tc.tile_pool(name="etp", bufs=16) as etp,
            tc.tile_pool(name="aop", bufs=8) as aop,
            tc.tile_pool(name="recp", bufs=4) as recp,
            tc.tile_pool(name="aotsb", bufs=5) as aotsb,
            tc.tile_pool(name="obp", bufs=8) as obp,
            tc.tile_pool(name="qkp", bufs=2, space="PSUM") as qkp,
            tc.tile_pool(name="ppp", bufs=1, space="PSUM") as ppp,
            tc.tile_pool(name="wap", bufs=2, space="PSUM") as wap,
            tc.tile_pool(name="avp", bufs=1, space="PSUM") as avp,
        ):
            # ---------------- prologue: DMAs + constants ----------------
            # Wq with the bias packed as column 512: one DMA, no separate
            # wqb transfer gating the first bias-add
            wq = singles.tile([128, 513], bf16, tag="wq")
            nc.sync.dma_start(out=wq, in_=WqB[:, :])
            # tensor_scalar needs an f32 scalar operand: unpack the bias col
            wqbf = singles.tile([128, 1], f32, tag="wqbf")
            nc.vector.tensor_copy(wqbf, wq[:, 512:513])

            xs = singles.tile([128, 4, S], bf16, tag="xs")
            # qb0 in two halves so qproj(0) can start sooner
            nc.sync.dma_start(out=xs[:, :, 0:256], in_=xB[:, :, 0:256])
            nc.sync.dma_start(out=xs[:, :, 256:512], in_=xB[:, :, 256:512])
            nc.sync.dma_start(out=xs[:, :, 512:1024], in_=xB[:, :, 512:1024])
            woT = singles.tile([128, HID], bf16, tag="woT")
            nc.sync.dma_start(out=woT, in_=WoT[:, :])
            nc.sync.dma_start(out=xs[:, :, 1024:1536], in_=xB[:, :, 1024:1536])
            nc.sync.dma_start(out=xs[:, :, 1536:2048], in_=xB[:, :, 1536:2048])

            # identities + causal mask first (gpsimd) so PE warmup can start
            identf = singles.tile([128, 64], f32, tag="identf")
            nc.gpsimd.memset(identf, 1.0)
            for p0 in (0, 64):
                nc.gpsimd.affine_select(
                    out=identf[p0 : p0 + 64, :], in_=identf[p0 : p0 + 64, :],
                    compare_op=ALU.is_equal,
                    fill=0.0, base=0, pattern=[[-1, 64]], channel_multiplier=1,
                )

            # exp ACT table preload while DMAs stream
            preld = singles.tile([32, 32], f32, tag="preld")
            nc.vector.memset(preld, 0.0)
            nc.scalar.activation(out=preld, in_=preld, func=Exp, scale=1.0)

            # dependency-free PE warmup: beats the p-state ramp so qproj(0)
            # runs at full clock as soon as its DMAs land
            warm = ppp.tile([128, SB], f32, tag="pp", name="warm")
            for j in range(N_WARMUP):
                nc.tensor.transpose(
                    warm[0:64, 64 * (j % 8) : 64 * (j % 8) + 64],
                    identf[0:64, :], identf[0:64, :],
                )

            ident64 = singles.tile([128, 64], bf16, tag="ident64")
            nc.vector.tensor_copy(ident64, identf)
            identb = singles.tile([128, 128], bf16, tag="identb")
            nc.gpsimd.memset(identb, 1.0)
            nc.gpsimd.affine_select(
                out=identb, in_=identb, compare_op=ALU.is_equal,
                fill=0.0, base=0, pattern=[[-1, 128]], channel_multiplier=1,
            )
            # trib[k, q] = 1 if k <= q else 0   (iota = q - k >= 0)
            trib = singles.tile([128, 128], bf16, tag="trib")
            nc.gpsimd.memset(trib, 1.0)
            nc.gpsimd.affine_select(
                out=trib, in_=trib, compare_op=ALU.is_ge,
                fill=0.0, base=0, pattern=[[1, 128]], channel_multiplier=-1,
            )

            qT = singles.tile([128, S], bf16, tag="qT")
            v_sb = [
                singles.tile([128, 16, 65], bf16, tag=f"v{h}", name=f"v{h}")
                for h in range(2)
            ]
            for h in range(2):
                nc.gpsimd.memset(v_sb[h][:, :, 64:65], 1.0)

            # state shared across the emission helpers
            et_map = {}    # (h, qb, kc) -> (et_tile, col_of_qc0)
            pending_masks = {}  # (h, qb, qc_local) -> (et_tile, col)
            ao_tiles = {}  # (h, qc_local) -> ao tile (bf16 [128, 64])
            av_cur = {}    # h -> av psum tile
            aot_ps = {}    # (h, qb) -> psum tile [64, 4, 128] bf16
            aot_sb = {}    # qb -> sbuf tile [128, 4, 128] bf16

            # ---------------- emission helpers ----------------
            def qproj(qb, halves=False):
                s0 = qb * SB
                qp = ppp.tile([128, SB], f32, tag="pp", name=f"qp{qb}")
                parts = ((0, 256), (256, 512)) if halves else ((0, SB),)
                for c0, c1 in parts:
                    for i in range(4):
                        nc.tensor.matmul(
                            qp[:, c0:c1], lhsT=wq[:, 128 * i : 128 * i + 128],
                            rhs=xs[:, i, s0 + c0 : s0 + c1],
                            start=(i == 0), stop=(i == 3),
                        )
                    nc.vector.tensor_scalar_add(
                        qT[:, s0 + c0 : s0 + c1], qp[:, c0:c1], wqbf
                    )


            def vprep(h, qb):
                hp = 64 * h
                vt = ppp.tile([128, 4, 64], bf16, tag="pp", name=f"vt{h}_{qb}")
                for j in range(4):
                    t0 = 128 * (4 * qb + j)
                    nc.tensor.transpose(
                        vt[:, j, :], qT[hp : hp + 64, t0 : t0 + 128],
                        ident64[hp : hp + 64, :],
                    )
                nc.vector.tensor_copy(v_sb[h][:, 4 * qb : 4 * qb + 4, 0:64], vt)

            def qk_group(h, qb, chunks, expw, masks, exp_splits=None,
                         defer_masks=True):
                """chunks: [(kc, coff, qoff, N)]; masks: [col] of tri blocks."""
                hp = 64 * h
                s0 = qb * SB
                qk = qkp.tile([128, 1024], f32, tag="qk", name="qk")
                et = etp.tile([128, 1024], bf16, tag="et", name="et")
                ranges = exp_splits or [(0, expw)]
                for kc, coff, qoff, n in chunks:
                    t0 = 128 * kc
                    nc.tensor.matmul(
                        qk[:, coff : coff + n],
                        lhsT=qT[hp : hp + 64, t0 : t0 + 128],
                        rhs=qT[hp : hp + 64, s0 + qoff : s0 + qoff + n],
                        start=True, stop=True,
                    )
                    et_map[(h, qb, kc)] = (et, coff - 128 * (qoff // 128))
                    # exp as soon as the covering chunk(s) are in psum
                    while ranges and ranges[0][1] <= coff + n:
                        e0, e1 = ranges.pop(0)
                        nc.scalar.activation(
                            out=et[:, e0:e1], in_=qk[:, e0:e1],
                            func=Exp, scale=SCALE,
                        )
                for e0, e1 in ranges:
                    nc.scalar.activation(
                        out=et[:, e0:e1], in_=qk[:, e0:e1], func=Exp, scale=SCALE
                    )
                for qc_l, mc in masks:
                    if defer_masks:
                        pending_masks[(h, qb, qc_l)] = (et, mc)
                    else:
                        nc.vector.tensor_mul(
                            et[:, mc : mc + 128], et[:, mc : mc + 128], trib
                        )

            def unit_groups(h, qb, split_first=False, defer_masks=True):
                k0 = 4 * qb
                gs = []
                for ke in range(0, k0, 2):  # off-diagonal pairs, full width
                    gs.append(
                        lambda ke=ke: qk_group(
                            h, qb,
                            [(ke, 0, 0, 512), (ke + 1, 512, 0, 512)],
                            1024, [],
                        )
                    )
                ch0 = (
                    [(k0, 0, 0, 256), (k0, 256, 256, 256)]
                    if split_first else [(k0, 0, 0, 512)]
                )
                splits = [(0, 256), (256, 896)] if split_first else None
                gs.append(  # diagonal pack A: kc0 (N=512) + kc1 (N=384)
                    lambda: qk_group(
                        h, qb, ch0 + [(k0 + 1, 512, 128, 384)], 896,
                        [(0, 0), (1, 512)],
                        exp_splits=splits, defer_masks=defer_masks,
                    )
                )
                gs.append(  # diagonal pack B: kc2 (N=256) + kc3 (N=128)
                    lambda: qk_group(
                        h, qb,
                        [(k0 + 2, 0, 256, 256), (k0 + 3, 256, 384, 128)],
                        384, [(2, 0), (3, 256)], defer_masks=defer_masks,
                    )
                )
                return gs

            def av_item(h, qb, qc_local, kc_from=0, kc_to=None):
                qc = 4 * qb + qc_local
                if kc_to is None:
                    kc_to = qc + 1
                if qc_local == 0 and kc_from == 0:
                    av_cur[h] = avp.tile(
                        [128, 4, 65], f32, tag="av", name=f"av{h}{qb}"
                    )
                av = av_cur[h]
                if kc_to > qc:  # this call includes the diagonal chunk
                    pm = pending_masks.pop((h, qb, qc_local), None)
                    if pm is not None:
                        met, mc = pm
                        nc.vector.tensor_mul(
                            met[:, mc : mc + 128], met[:, mc : mc + 128], trib
                        )
                for kc in range(kc_from, min(kc_to, qc + 1)):
                    et, c0 = et_map[(h, qb, kc)]
                    nc.tensor.matmul(
                        av[:, qc_local, :],
                        lhsT=et[:, c0 + 128 * qc_local : c0 + 128 * qc_local + 128],
                        rhs=v_sb[h][:, kc, :],
                        start=(kc == 0), stop=(kc == qc),
                    )

            def div_item(h, qc_local, rec=None):
                """ao = av[:, qc, 0:64] * (1 / av[:, qc, 64]) -> bf16."""
                av = av_cur[h]
                if rec is None:  # per-qc reciprocal (drain path)
                    rec = recp.tile([128, 1, 1], f32, tag="rec", name="rec1")
                    nc.vector.reciprocal(rec, av[:, qc_local, 64:65])
                    rslice = rec[:, 0, :]
                else:
                    rslice = rec[:, qc_local, :]
                ao = aop.tile([128, 64], bf16, tag="ao", name="ao")
                nc.vector.tensor_scalar_mul(ao, av[:, qc_local, 0:64], rslice)
                ao_tiles[(h, qc_local)] = ao

            def norm_item(h, qb):
                rec = recp.tile([128, 4, 1], f32, tag="rec", name="rec4")
                nc.vector.reciprocal(rec, av_cur[h][:, :, 64:65])
                for qc_local in range(4):
                    div_item(h, qc_local, rec=rec)

            def t_item(h, qb):
                ps = wap.tile([64, 4, 128], bf16, tag="wap", name=f"aot{h}{qb}")
                aot_ps[(h, qb)] = ps
                for qc_local in range(4):
                    nc.tensor.transpose(
                        ps[:, qc_local, :], ao_tiles[(h, qc_local)], identb
                    )

            def aot_copy(qb, h):
                if qb not in aot_sb:
                    aot_sb[qb] = aotsb.tile(
                        [128, 4, 128], bf16, tag="aotsb", name=f"aotsb{qb}"
                    )
                sb = aot_sb[qb]
                nc.vector.tensor_copy(
                    sb[64 * h : 64 * h + 64, :, :], aot_ps[(h, qb)]
                )

            def w_item(qb, qc_local, copy_eng, dma_eng, wp=None):
                if wp is None:
                    wp = wap.tile(
                        [128, SB], f32, tag="wap", name=f"wp{qb}{qc_local}"
                    )
                nc.tensor.matmul(
                    wp, lhsT=aot_sb[qb][:, qc_local, :], rhs=woT,
                    start=True, stop=True,
                )
                ob = obp.tile([128, SB], bf16, tag="ob", name="ob")
                if hasattr(copy_eng, "tensor_copy"):
                    copy_eng.tensor_copy(ob, wp)
                else:
                    copy_eng.copy(ob, wp)  # scalar engine (ACT)
                r0 = 512 * qb + 128 * qc_local
                dma_eng.dma_start(out=out_part[r0 : r0 + 128, :], in_=ob)

            def w_items(qb):
                # ob copies must read PSUM: only DVE/ACT can. DMAs alternate
                # between the HWDGE (sync) and SWDGE (gpsimd) queues.
                out = []
                for qc_local in range(4):
                    out.append(lambda q=qb, c=qc_local: w_item(q, c, nc.vector, nc.sync))
                return out

            def av_norm_t(h, qb):
                return [lambda c=c: av_item(h, qb, c) for c in range(4)] + [
                    lambda: norm_item(h, qb),
                    lambda: t_item(h, qb),
                ]

            def drain_steps(qb, engines, use_qkp=False, pre=False):
                """Software-pipelined drain of head 1 of block qb: the four
                per-qc chains (AV -> divide -> transpose -> copy -> Wo -> DMA)
                emitted as a diagonal wavefront so the in-order engines never
                wait a full chain. Returns a list of emit-thunks (steps); with
                pre=True the off-diagonal AV accumulation is split out as four
                leading steps that only need the unit's off-diagonal exps."""
                h = 1
                ps_t = {}

                def av_s(qc):
                    if not pre:
                        return lambda: av_item(h, qb, qc)
                    return lambda: av_item(h, qb, qc, kc_from=4 * qb)

                def div_s(qc):
                    return lambda: div_item(h, qc)

                def t_s(qc):
                    def f():
                        ps = wap.tile(
                            [64, 1, 128], bf16, tag="wap", name=f"aotd{qb}{qc}"
                        )
                        ps_t[qc] = ps
                        nc.tensor.transpose(
                            ps[:, 0, :], ao_tiles[(h, qc)], identb
                        )
                    return f

                def cp_s(qc):
                    def f():
                        nc.vector.tensor_copy(
                            aot_sb[qb][64:128, qc, :], ps_t[qc][:, 0, :]
                        )
                    return f

                wp_pairs = {}

                def w_s(qc, use_qkp):
                    ce, de = engines[qc]

                    def f():
                        wp = None
                        if use_qkp:
                            # the QK stream is done: its psum banks are free
                            # and a [128, 1024] buf holds two Wo outputs
                            if qc % 2 == 0:
                                wp_pairs[qc // 2] = qkp.tile(
                                    [128, 1024], f32, tag="qk", name=f"wpd{qc}"
                                )
                            pair = wp_pairs[qc // 2]
                            wp = pair[:, 512 * (qc % 2) : 512 * (qc % 2) + 512]
                        w_item(qb, qc, ce, de, wp=wp)

                    return f

                uq = use_qkp
                waves = [
                    [av_s(0)],
                    [av_s(1), div_s(0)],
                    [av_s(2), div_s(1), t_s(0)],
                    [av_s(3), div_s(2), cp_s(0), t_s(1)],
                    [div_s(3), cp_s(1), t_s(2), w_s(0, uq)],
                    [cp_s(2), t_s(3), w_s(1, uq)],
                    [cp_s(3), w_s(2, uq)],
                    [w_s(3, uq)],
                ]

                def run(wave):
                    return lambda: [f() for f in wave]

                steps = [run(w) for w in waves]
                if pre:
                    steps = [
                        lambda qc=qc: av_item(h, qb, qc, kc_to=4 * qb)
                        for qc in range(4)
                    ] + steps
                return steps

            def emit_unit(h, qb, hk, split_first=False, defer_masks=True):
                gs = unit_groups(h, qb, split_first=split_first,
                                 defer_masks=defer_masks)
                hk = list(hk)
                for g in gs:
                    g()
                    if hk:
                        hk.pop(0)()
                for item in hk:
                    item()
                return []

            def interleave(ws, avs, rest):
                """Alternate stall-prone W chains with cheap AV filler so an
                in-order PE never has two wp-waits back to back."""
                out = []
                for i in range(max(len(ws), len(avs))):
                    if i < len(ws):
                        out.append(ws[i])
                    if i < len(avs):
                        out.append(avs[i])
                return out + rest

            # ---------------- main schedule ----------------
            # unit order: (0,0) (1,0) (0,1) (1,1) (0,2) (0,3) (1,3)
            #             (1,2)+[qb3 drain] [qb2 drain]
            qproj(0, halves=True)
            emit_unit(
                0, 0, [lambda: vprep(0, 0), lambda: vprep(1, 0)],
                split_first=True,
            )
            emit_unit(
                1, 0,
                [lambda: qproj(1)] + av_norm_t(0, 0)
                + [lambda: aot_copy(0, 0)],
                split_first=True,
            )
            emit_unit(
                0, 1,
                av_norm_t(1, 0)
                + [lambda: aot_copy(0, 1),
                   lambda: vprep(0, 1), lambda: vprep(1, 1)],
            )
            emit_unit(
                1, 1,
                [lambda: qproj(2)]
                + interleave(
                    w_items(0),
                    [lambda c=c: av_item(0, 1, c) for c in range(4)],
                    [lambda: norm_item(0, 1), lambda: t_item(0, 1),
                     lambda: aot_copy(1, 0)],
                ),
            )
            emit_unit(
                0, 2,
                [lambda: qproj(3)] + av_norm_t(1, 1)
                + [lambda: aot_copy(1, 1),
                   lambda: vprep(0, 2), lambda: vprep(1, 2)],
            )
            emit_unit(
                0, 3,
                interleave(
                    w_items(1),
                    [lambda c=c: av_item(0, 2, c) for c in range(4)],
                    [lambda: norm_item(0, 2), lambda: t_item(0, 2),
                     lambda: aot_copy(2, 0),
                     lambda: vprep(0, 3), lambda: vprep(1, 3)],
                ),
            )
            emit_unit(
                1, 3,
                av_norm_t(0, 3) + [lambda: aot_copy(3, 0)],
            )
            # qb3 drain wavefront interleaves with unit (1,2)'s QK groups so
            # the ACT exp stream stays fed while qb3 drains
            d3 = drain_steps(3, {0: (nc.vector, nc.sync),
                                 1: (nc.vector, nc.sync),
                                 2: (nc.vector, nc.sync),
                                 3: (nc.vector, nc.sync)})
            d3p = [lambda a=a, b=b: (a(), b()) for a, b in zip(d3[0::2], d3[1::2])]
            emit_unit(1, 2, d3p, defer_masks=False)
            # final drain wavefront: qb2, head 1 (wp pairs in the freed qk
            # psum so no Wo waits an output copy)
            d2 = drain_steps(2, {0: (nc.scalar, nc.sync),
                                 1: (nc.scalar, nc.sync),
                                 2: (nc.scalar, nc.sync),
                                 3: (nc.scalar, nc.sync)},
                             use_qkp=True, pre=False)
            for step in d2:
                step()

    nc.finalize()
    return nc


_NC_CACHE = None


def _get_nc():
    global _NC_CACHE
    if _NC_CACHE is None:
        _NC_CACHE = build_nc()
    return _NC_CACHE


def make_in_maps(x, Wq_w, Wq_b, Wo_w):
    x = np.asarray(x, dtype=np.float32)
    Wq_w = np.asarray(Wq_w, dtype=np.float32)
    Wq_b = np.asarray(Wq_b, dtype=np.float32)
    Wo_w = np.asarray(Wo_w, dtype=np.float32)
    in_maps = []
    for c in range(N_CORES):
        b, hp = divmod(c, 4)
        dq = slice(128 * hp, 128 * (hp + 1))
        xBc = np.ascontiguousarray(x[b].T.reshape(4, 128, S).transpose(1, 0, 2))
        WqBc = np.ascontiguousarray(
            Wq_w[dq, :].T.reshape(4, 128, 128).transpose(1, 0, 2)
        )
        WqBp = np.concatenate(
            [WqBc.reshape(128, 512), Wq_b[dq].reshape(128, 1)], axis=1
        )
        in_maps.append({
            "xB": xBc.astype(ml_dtypes.bfloat16),
            "WqB": np.ascontiguousarray(WqBp).astype(ml_dtypes.bfloat16),
            "WoT": np.ascontiguousarray(Wo_w[:, dq].T).astype(ml_dtypes.bfloat16),
        })
    return in_maps


def kernel(x, mask, Wq_w, Wq_b, Wo_w, Wo_b, **_):
    nc = _get_nc()
    in_maps = make_in_maps(x, Wq_w, Wq_b, Wo_w)
    res = run_bass_kernel_spmd(nc, in_maps, core_ids=list(range(N_CORES)))
    Wo_b = np.asarray(Wo_b, dtype=np.float32)
    out = np.empty((B, S, HID), dtype=np.float32)
    for b in range(B):
        acc = np.asarray(res.results[4 * b]["out_part"], dtype=np.float32)
        for c in range(4 * b + 1, 4 * b + 4):
            acc = acc + np.asarray(res.results[c]["out_part"], dtype=np.float32)
        out[b] = acc + Wo_b[None, :]
    return out


# revision 54
# speedup vs baseline: 1.0005x; 1.0005x over previous
"""Trainium2 Bass kernel for nn_Attention_36404142801494.

Fused causal self-attention (q=k=v=Wq(x)) + output projection, sharded over
8 NeuronCores: data-parallel on batch (B=2 -> 2 groups of 4 cores), tensor-
parallel on heads (8 heads -> 2 heads/core, dq = 128 hidden dims/core) with a
column-split Wq and a row-split Wo. Each core returns a partial [S, HID]
output (bf16); the host sums the 4 partials per batch and adds the Wo bias.

Per-core structure (keyed to the TimelineSim cost model, where a matmul costs
out_free_size x cycles_per_row and LDWEIGHTS is free):
  - qT [d=128, s] f32r via Wq matmuls (moving = x blocks, N=512/256).
  - QK emits scoresT tiles [k=128, q<=512] per key-chunk, causally trimmed;
    f32r keeps 1 cyc/row at N>=256 (diagonal chunks padded to N=256 min).
  - exp on ACT into bf16 et tiles; diagonal 128x128 blocks masked by a
    lower-triangular bf16 mask multiply on DVE (mask generated on device).
  - AV runs TRANSPOSED: stationary = et 128x128 block, moving = V chunk
    [k=128, 65] bf16 (col 64 = ones) -> av psum [q=128, 65], N=65/matmul.
    Col 64 accumulates the softmax denominator for free; normalization is a
    per-partition reciprocal + tensor_scalar multiply (no DRAM bounce).
  - ao [q, d] bf16 is PE-transposed (bf16 identity) to aoT [d, q]; both
    heads stack to [128, q] so Wo is one K=128 matmul per 128-query chunk
    (moving = WoT [128 d, 512 c] bf16, N=512).
  - Output partials DMA out as bf16 [2048, 512], split across HWDGE and
    SWDGE queues.

Schedule: unit (h, qb) = the QK/exp stream for one head x query block.
Units run (0,0),(1,0),(0,1),(1,1),(0,2),(0,3),(1,3),(1,2); qproj of the
next block is emitted as housekeeping inside an earlier unit so the ACT
engine keeps exp backlog across block boundaries. Heads 1 of qb3 and qb2
drain through software-pipelined per-query-chunk wavefronts (AV -> recip/
mul -> transpose -> copy -> Wo -> DMA); the qb3 drain overlaps unit (1,2)'s
exp stream and the final qb2 drain reuses the freed QK psum banks. A block
of dependency-free PE warmup transposes before qproj(0) beats the p-state
ramp while the first DMAs land. Inputs x/Wq/Wo stream in as bf16.

Everything is hardcoded for B=2, S=2048, HID=512, NH=8, HD=64.
"""

import sys

sys.path.insert(0, "/opt/trn_rl_repo")

import numpy as np
import ml_dtypes

import concourse.bass as bass
import concourse.bacc as bacc
import concourse.tile as tile
import concourse.mybir as mybir
from concourse.bass_utils import run_bass_kernel_spmd

f32 = mybir.dt.float32
f32r = mybir.dt.float32r
bf16 = mybir.dt.bfloat16

B, S, HID = 2, 2048, 512
NH, HD = 8, 64
N_CORES = 8
SB = 512
SCALE = 1.0 / np.sqrt(HD)
N_WARMUP = 22

Exp = mybir.ActivationFunctionType.Exp
ALU = mybir.AluOpType


def build_nc():
    nc = bacc.Bacc(None, target_bir_lowering=False)

    # host pre-arranged layouts (see make_in_maps):
    #   xB[p, i, s]  = x[b].T[128*i + p, s]
    #   WqB[p, i, c] = Wq_w[dq, :].T[128*i + p, c]
    #   WoT[d, c]    = Wo_w[:, dq].T[d, c]
    xB = nc.dram_tensor("xB", [128, 4, S], bf16, kind="ExternalInput")
    WqB = nc.dram_tensor("WqB", [128, 513], bf16, kind="ExternalInput")
    WoT = nc.dram_tensor("WoT", [128, HID], bf16, kind="ExternalInput")
    out_part = nc.dram_tensor("out_part", [S, HID], bf16, kind="ExternalOutput")

    with tile.TileContext(nc) as tc:
        with (
            tc.tile_pool(name="singles", bufs=1) as singles,
            tc.tile_pool(name="etp", bufs=17) as etp,
            tc.tile_pool(name="aop", bufs=6) as aop,
            tc.tile_pool(name="recp", bufs=2) as recp,
            tc.tile_pool(name="aotsb", bufs=4) as aotsb,
            tc.tile_pool(name="obp", bufs=4) as obp,
            tc.tile_pool(name="qkp", bufs=2, space="PSUM") as qkp,
            tc.tile_pool(name="ppp", bufs=1, space="PSUM") as ppp,
            tc.tile_pool(name="wap", bufs=2, space="PSUM") as wap,
            tc.tile_pool(name="avp", bufs=1, space="PSUM") as avp,
        ):
            # ---------------- prologue: DMAs + constants ----------------
            # Wq with the bias packed as column 512: one DMA, no separate
            # wqb transfer gating the first bias-add
            wq = singles.tile([128, 513], bf16, tag="wq")
            nc.sync.dma_start(out=wq, in_=WqB[:, :])
            # tensor_scalar needs an f32 scalar operand: unpack the bias col
            wqbf = singles.tile([128, 1], f32, tag="wqbf")
            nc.vector.tensor_copy(wqbf, wq[:, 512:513])

            xs = singles.tile([128, 4, S], bf16, tag="xs")
            # qb0 in two halves so qproj(0) can start sooner
            nc.sync.dma_start(out=xs[:, :, 0:256], in_=xB[:, :, 0:256])
            nc.sync.dma_start(out=xs[:, :, 256:512], in_=xB[:, :, 256:512])
            nc.sync.dma_start(out=xs[:, :, 512:1024], in_=xB[:, :, 512:1024])
            woT = singles.tile([128, HID], bf16, tag="woT")
            nc.sync.dma_start(out=woT, in_=WoT[:, :])
            nc.sync.dma_start(out=xs[:, :, 1024:1536], in_=xB[:, :, 1024:1536])
            nc.sync.dma_start(out=xs[:, :, 1536:2048], in_=xB[:, :, 1536:2048])

            # identities + causal mask first (gpsimd) so PE warmup can start
            identf = singles.tile([128, 64], f32, tag="identf")
            nc.gpsimd.memset(identf, 1.0)
            for p0 in (0, 64):
                nc.gpsimd.affine_select(
                    out=identf[p0 : p0 + 64, :], in_=identf[p0 : p0 + 64, :],
                    compare_op=ALU.is_equal,
                    fill=0.0, base=0, pattern=[[-1, 64]], channel_multiplier=1,
                )

            # exp ACT table preload while DMAs stream
            preld = singles.tile([32, 32], f32, tag="preld")
            nc.vector.memset(preld, 0.0)
            nc.scalar.activation(out=preld, in_=preld, func=Exp, scale=1.0)

            # dependency-free PE warmup: beats the p-state ramp so qproj(0)
            # runs at full clock as soon as its DMAs land
            warm = ppp.tile([128, SB], f32, tag="pp", name="warm")
            for j in range(N_WARMUP):
                nc.tensor.transpose(
                    warm[0:64, 64 * (j % 8) : 64 * (j % 8) + 64],
                    identf[0:64, :], identf[0:64, :],
                )

            ident64 = singles.tile([128, 64], bf16, tag="ident64")
            nc.vector.tensor_copy(ident64, identf)
            identb = singles.tile([128, 128], bf16, tag="identb")
            nc.gpsimd.memset(identb, 1.0)
            nc.gpsimd.affine_select(
                out=identb, in_=identb, compare_op=ALU.is_equal,
                fill=0.0, base=0, pattern=[[-1, 128]], channel_multiplier=1,
            )
            # trib[k, q] = 1 if k <= q else 0   (iota = q - k >= 0)
            trib = singles.tile([128, 128], bf16, tag="trib")
            nc.gpsimd.memset(trib, 1.0)
            nc.gpsimd.affine_select(
                out=trib, in_=trib, compare_op=ALU.is_ge,
                fill=0.0, base=0, pattern=[[1, 128]], channel_multiplier=-1,
            )

            qT = singles.tile([128, S], bf16, tag="qT")
            v_sb = [
                singles.tile([128, 16, 65], bf16, tag=f"v{h}", name=f"v{h}")
                for h in range(2)
            ]
            for h in range(2):
                nc.gpsimd.memset(v_sb[h][:, :, 64:65], 1.0)

            # state shared across the emission helpers
            et_map = {}    # (h, qb, kc) -> (et_tile, col_of_qc0)
            pending_masks = {}  # (h, qb, qc_local) -> (et_tile, col)
            ao_tiles = {}  # (h, qc_local) -> ao tile (bf16 [128, 64])
            av_cur = {}    # h -> av psum tile
            aot_ps = {}    # (h, qb) -> psum tile [64, 4, 128] bf16
            aot_sb = {}    # qb -> sbuf tile [128, 4, 128] bf16

            # ---------------- emission helpers ----------------
            def qproj(qb, halves=False):
                s0 = qb * SB
                qp = ppp.tile([128, SB], f32, tag="pp", name=f"qp{qb}")
                parts = ((0, 256), (256, 512)) if halves else ((0, SB),)
                for c0, c1 in parts:
                    for i in range(4):
                        nc.tensor.matmul(
                            qp[:, c0:c1], lhsT=wq[:, 128 * i : 128 * i + 128],
                            rhs=xs[:, i, s0 + c0 : s0 + c1],
                            start=(i == 0), stop=(i == 3),
                        )
                    nc.vector.tensor_scalar_add(
                        qT[:, s0 + c0 : s0 + c1], qp[:, c0:c1], wqbf
                    )


            def vprep(h, qb):
                hp = 64 * h
                vt = ppp.tile([128, 4, 64], bf16, tag="pp", name=f"vt{h}_{qb}")
                for j in range(4):
                    t0 = 128 * (4 * qb + j)
                    nc.tensor.transpose(
                        vt[:, j, :], qT[hp : hp + 64, t0 : t0 + 128],
                        ident64[hp : hp + 64, :],
                    )
                nc.vector.tensor_copy(v_sb[h][:, 4 * qb : 4 * qb + 4, 0:64], vt)

            def qk_group(h, qb, chunks, expw, masks, exp_splits=None,
                         defer_masks=True):
                """chunks: [(kc, coff, qoff, N)]; masks: [col] of tri blocks."""
                hp = 64 * h
                s0 = qb * SB
                qk = qkp.tile([128, 1024], f32, tag="qk", name="qk")
                et = etp.tile([128, 1024], bf16, tag="et", name="et")
                ranges = exp_splits or [(0, expw)]
                for kc, coff, qoff, n in chunks:
                    t0 = 128 * kc
                    nc.tensor.matmul(
                        qk[:, coff : coff + n],
                        lhsT=qT[hp : hp + 64, t0 : t0 + 128],
                        rhs=qT[hp : hp + 64, s0 + qoff : s0 + qoff + n],
                        start=True, stop=True,
                    )
                    et_map[(h, qb, kc)] = (et, coff - 128 * (qoff // 128))
                    # exp as soon as the covering chunk(s) are in psum
                    while ranges and ranges[0][1] <= coff + n:
                        e0, e1 = ranges.pop(0)
                        nc.scalar.activation(
                            out=et[:, e0:e1], in_=qk[:, e0:e1],
                            func=Exp, scale=SCALE,
                        )
                for e0, e1 in ranges:
                    nc.scalar.activation(
                        out=et[:, e0:e1], in_=qk[:, e0:e1], func=Exp, scale=SCALE
                    )
                for qc_l, mc in masks:
                    if defer_masks:
                        pending_masks[(h, qb, qc_l)] = (et, mc)
                    else:
                        nc.vector.tensor_mul(
                            et[:, mc : mc + 128], et[:, mc : mc + 128], trib
                        )

            def unit_groups(h, qb, split_first=False, defer_masks=True):
                k0 = 4 * qb
                gs = []
                for ke in range(0, k0, 2):  # off-diagonal pairs, full width
                    gs.append(
                        lambda ke=ke: qk_group(
                            h, qb,
                            [(ke, 0, 0, 512), (ke + 1, 512, 0, 512)],
                            1024, [],
                        )
                    )
                ch0 = (
                    [(k0, 0, 0, 256), (k0, 256, 256, 256)]
                    if split_first else [(k0, 0, 0, 512)]
                )
                splits = [(0, 256), (256, 896)] if split_first else None
                gs.append(  # diagonal pack A: kc0 (N=512) + kc1 (N=384)
                    lambda: qk_group(
                        h, qb, ch0 + [(k0 + 1, 512, 128, 384)], 896,
                        [(0, 0), (1, 512)],
                        exp_splits=splits, defer_masks=defer_masks,
                    )
                )
                gs.append(  # diagonal pack B: kc2 (N=256) + kc3 (N=128)
                    lambda: qk_group(
                        h, qb,
                        [(k0 + 2, 0, 256, 256), (k0 + 3, 256, 384, 128)],
                        384, [(2, 0), (3, 256)], defer_masks=defer_masks,
                    )
                )
                return gs

            def av_item(h, qb, qc_local, kc_from=0, kc_to=None):
                qc = 4 * qb + qc_local
                if kc_to is None:
                    kc_to = qc + 1
                if qc_local == 0 and kc_from == 0:
                    av_cur[h] = avp.tile(
                        [128, 4, 65], f32, tag="av", name=f"av{h}{qb}"
                    )
                av = av_cur[h]
                if kc_to > qc:  # this call includes the diagonal chunk
                    pm = pending_masks.pop((h, qb, qc_local), None)
                    if pm is not None:
                        met, mc = pm
                        nc.vector.tensor_mul(
                            met[:, mc : mc + 128], met[:, mc : mc + 128], trib
                        )
                for kc in range(kc_from, min(kc_to, qc + 1)):
                    et, c0 = et_map[(h, qb, kc)]
                    nc.tensor.matmul(
                        av[:, qc_local, :],
                        lhsT=et[:, c0 + 128 * qc_local : c0 + 128 * qc_local + 128],
                        rhs=v_sb[h][:, kc, :],
                        start=(kc == 0), stop=(kc == qc),
                    )

            def div_item(h, qc_local, rec=None):
                """ao = av[:, qc, 0:64] * (1 / av[:, qc, 64]) -> bf16."""
                av = av_cur[h]
                if rec is None:  # per-qc reciprocal (drain path)
                    rec = recp.tile([128, 1, 1], f32, tag="rec", name="rec1")
                    nc.vector.reciprocal(rec, av[:, qc_local, 64:65])
                    rslice = rec[:, 0, :]
                else:
                    rslice = rec[:, qc_local, :]
                ao = aop.tile([128, 64], bf16, tag="ao", name="ao")
                nc.vector.tensor_scalar_mul(ao, av[:, qc_local, 0:64], rslice)
                ao_tiles[(h, qc_local)] = ao

            def norm_item(h, qb):
                rec = recp.tile([128, 4, 1], f32, tag="rec", name="rec4")
                nc.vector.reciprocal(rec, av_cur[h][:, :, 64:65])
                for qc_local in range(4):
                    div_item(h, qc_local, rec=rec)

            def t_item(h, qb):
                ps = wap.tile([64, 4, 128], bf16, tag="wap", name=f"aot{h}{qb}")
                aot_ps[(h, qb)] = ps
                for qc_local in range(4):
                    nc.tensor.transpose(
                        ps[:, qc_local, :], ao_tiles[(h, qc_local)], identb
                    )

            def aot_copy(qb, h):
                if qb not in aot_sb:
                    aot_sb[qb] = aotsb.tile(
                        [128, 4, 128], bf16, tag="aotsb", name=f"aotsb{qb}"
                    )
                sb = aot_sb[qb]
                nc.vector.tensor_copy(
                    sb[64 * h : 64 * h + 64, :, :], aot_ps[(h, qb)]
                )

            def w_item(qb, qc_local, copy_eng, dma_eng, wp=None):
                if wp is None:
                    wp = wap.tile(
                        [128, SB], f32, tag="wap", name=f"wp{qb}{qc_local}"
                    )
                nc.tensor.matmul(
                    wp, lhsT=aot_sb[qb][:, qc_local, :], rhs=woT,
                    start=True, stop=True,
                )
                ob = obp.tile([128, SB], bf16, tag="ob", name="ob")
                if hasattr(copy_eng, "tensor_copy"):
                    copy_eng.tensor_copy(ob, wp)
                else:
                    copy_eng.copy(ob, wp)  # scalar engine (ACT)
                r0 = 512 * qb + 128 * qc_local
                dma_eng.dma_start(out=out_part[r0 : r0 + 128, :], in_=ob)

            def w_items(qb):
                # ob copies must read PSUM: only DVE/ACT can. DMAs alternate
                # between the HWDGE (sync) and SWDGE (gpsimd) queues.
                out = []
                for qc_local in range(4):
                    out.append(lambda q=qb, c=qc_local: w_item(q, c, nc.vector, nc.sync))
                return out

            def av_norm_t(h, qb):
                return [lambda c=c: av_item(h, qb, c) for c in range(4)] + [
                    lambda: norm_item(h, qb),
                    lambda: t_item(h, qb),
                ]

            def drain_steps(qb, engines, use_qkp=False, pre=False):
                """Software-pipelined drain of head 1 of block qb: the four
                per-qc chains (AV -> divide -> transpose -> copy -> Wo -> DMA)
                emitted as a diagonal wavefront so the in-order engines never
                wait a full chain. Returns a list of emit-thunks (steps); with
                pre=True the off-diagonal AV accumulation is split out as four
                leading steps that only need the unit's off-diagonal exps."""
                h = 1
                ps_t = {}

                def av_s(qc):
                    if not pre:
                        return lambda: av_item(h, qb, qc)
                    return lambda: av_item(h, qb, qc, kc_from=4 * qb)

                def div_s(qc):
                    return lambda: div_item(h, qc)

                def t_s(qc):
                    def f():
                        ps = wap.tile(
                            [64, 1, 128], bf16, tag="wap", name=f"aotd{qb}{qc}"
                        )
                        ps_t[qc] = ps
                        nc.tensor.transpose(
                            ps[:, 0, :], ao_tiles[(h, qc)], identb
                        )
                    return f

                def cp_s(qc):
                    def f():
                        nc.vector.tensor_copy(
                            aot_sb[qb][64:128, qc, :], ps_t[qc][:, 0, :]
                        )
                    return f

                wp_pairs = {}

                def w_s(qc, use_qkp):
                    ce, de = engines[qc]

                    def f():
                        wp = None
                        if use_qkp:
                            # the QK stream is done: its psum banks are free
                            # and a [128, 1024] buf holds two Wo outputs
                            if qc % 2 == 0:
                                wp_pairs[qc // 2] = qkp.tile(
                                    [128, 1024], f32, tag="qk", name=f"wpd{qc}"
                                )
                            pair = wp_pairs[qc // 2]
                            wp = pair[:, 512 * (qc % 2) : 512 * (qc % 2) + 512]
                        w_item(qb, qc, ce, de, wp=wp)

                    return f

                uq = use_qkp
                waves = [
                    [av_s(0)],
                    [av_s(1), div_s(0)],
                    [av_s(2), div_s(1), t_s(0)],
                    [av_s(3), div_s(2), cp_s(0), t_s(1)],
                    [div_s(3), cp_s(1), t_s(2), w_s(0, uq)],
                    [cp_s(2), t_s(3), w_s(1, uq)],
                    [cp_s(3), w_s(2, uq)],
                    [w_s(3, uq)],
                ]

                def run(wave):
                    return lambda: [f() for f in wave]

                steps = [run(w) for w in waves]
                if pre:
                    steps = [
                        lambda qc=qc: av_item(h, qb, qc, kc_to=4 * qb)
                        for qc in range(4)
                    ] + steps
                return steps

            def emit_unit(h, qb, hk, split_first=False, defer_masks=True):
                gs = unit_groups(h, qb, split_first=split_first,
                                 defer_masks=defer_masks)
                hk = list(hk)
                for g in gs:
                    g()
                    if hk:
                        hk.pop(0)()
                for item in hk:
                    item()
                return []

            def interleave(ws, avs, rest):
                """Alternate stall-prone W chains with cheap AV filler so an
                in-order PE never has two wp-waits back to back."""
                out = []
                for i in range(max(len(ws), len(avs))):
                    if i < len(ws):
                        out.append(ws[i])
                    if i < len(avs):
                        out.append(avs[i])
                return out + rest

            # ---------------- main schedule ----------------
            # unit order: (0,0) (1,0) (0,1) (1,1) (0,2) (0,3) (1,3)
            #             (1,2)+[qb3 drain] [qb2 drain]
            qproj(0, halves=True)
            emit_unit(
                0, 0, [lambda: vprep(0, 0), lambda: vprep(1, 0)],
                split_first=True,
            )
            emit_unit(
                1, 0,
                [lambda: qproj(1)] + av_norm_t(0, 0)
                + [lambda: aot_copy(0, 0)],
                split_first=True,
            )
            emit_unit(
                0, 1,
                av_norm_t(1, 0)
                + [lambda: aot_copy(0, 1),
                   lambda: vprep(0, 1), lambda: vprep(1, 1)],
            )
            emit_unit(
                1, 1,
                [lambda: qproj(2)]
                + interleave(
                    w_items(0),
                    [lambda c=c: av_item(0, 1, c) for c in range(4)],
                    [lambda: norm_item(0, 1), lambda: t_item(0, 1),
                     lambda: aot_copy(1, 0)],
                ),
            )
            emit_unit(
                0, 2,
                [lambda: qproj(3)] + av_norm_t(1, 1)
                + [lambda: aot_copy(1, 1),
                   lambda: vprep(0, 2), lambda: vprep(1, 2)],
            )
            emit_unit(
                0, 3,
                interleave(
                    w_items(1),
                    [lambda c=c: av_item(0, 2, c) for c in range(4)],
                    [lambda: norm_item(0, 2), lambda: t_item(0, 2),
                     lambda: aot_copy(2, 0),
                     lambda: vprep(0, 3), lambda: vprep(1, 3)],
                ),
            )
            emit_unit(
                1, 3,
                av_norm_t(0, 3) + [lambda: aot_copy(3, 0)],
            )
            # qb3 drain wavefront interleaves with unit (1,2)'s QK groups so
            # the ACT exp stream stays fed while qb3 drains
            d3 = drain_steps(3, {0: (nc.vector, nc.sync),
                                 1: (nc.vector, nc.sync),
                                 2: (nc.vector, nc.sync),
                                 3: (nc.vector, nc.sync)})
            d3p = [lambda a=a, b=b: (a(), b()) for a, b in zip(d3[0::2], d3[1::2])]
            emit_unit(1, 2, d3p, defer_masks=False)
            # final drain wavefront: qb2, head 1 (wp pairs in the freed qk
            # psum so no Wo waits an output copy)
            d2 = drain_steps(2, {0: (nc.scalar, nc.sync),
                                 1: (nc.scalar, nc.sync),
                                 2: (nc.scalar, nc.sync),
                                 3: (nc.scalar, nc.sync)},
                             use_qkp=True, pre=False)
            for step in d2:
                step()

    nc.finalize()
    return nc


_NC_CACHE = None


def _get_nc():
    global _NC_CACHE
    if _NC_CACHE is None:
        _NC_CACHE = build_nc()
    return _NC_CACHE


def make_in_maps(x, Wq_w, Wq_b, Wo_w):
    x = np.asarray(x, dtype=np.float32)
    Wq_w = np.asarray(Wq_w, dtype=np.float32)
    Wq_b = np.asarray(Wq_b, dtype=np.float32)
    Wo_w = np.asarray(Wo_w, dtype=np.float32)
    in_maps = []
    for c in range(N_CORES):
        b, hp = divmod(c, 4)
        dq = slice(128 * hp, 128 * (hp + 1))
        xBc = np.ascontiguousarray(x[b].T.reshape(4, 128, S).transpose(1, 0, 2))
        WqBc = np.ascontiguousarray(
            Wq_w[dq, :].T.reshape(4, 128, 128).transpose(1, 0, 2)
        )
        WqBp = np.concatenate(
            [WqBc.reshape(128, 512), Wq_b[dq].reshape(128, 1)], axis=1
        )
        in_maps.append({
            "xB": xBc.astype(ml_dtypes.bfloat16),
            "WqB": np.ascontiguousarray(WqBp).astype(ml_dtypes.bfloat16),
            "WoT": np.ascontiguousarray(Wo_w[:, dq].T).astype(ml_dtypes.bfloat16),
        })
    return in_maps


def kernel(x, mask, Wq_w, Wq_b, Wo_w, Wo_b, **_):
    nc = _get_nc()
    in_maps = make_in_maps(x, Wq_w, Wq_b, Wo_w)
    res = run_bass_kernel_spmd(nc, in_maps, core_ids=list(range(N_CORES)))
    Wo_b = np.asarray(Wo_b, dtype=np.float32)
    out = np.empty((B, S, HID), dtype=np.float32)
    for b in range(B):
        acc = np.asarray(res.results[4 * b]["out_part"], dtype=np.float32)
        for c in range(4 * b + 1, 4 * b + 4):
            acc = acc + np.asarray(res.results[c]["out_part"], dtype=np.float32)
        out[b] = acc + Wo_b[None, :]
    return out
